# revision 24
# baseline (speedup 1.0000x reference)
"""Trainium2 Bass kernel for nn_MultiHeadAttention (B=2,T=2048,D=1024,H=16,HD=64).

Sharding: 8 cores = 2 batches x 4 heads/core (tensor parallel over heads).
Each core: q/k/v projections for its 4 heads, RoPE, causal attention, and a
partial output projection (its heads' slice of Wp); host sums 4 partials/batch.

v2 structure (vs baseline): attention runs on 256-wide query chunks with a
double-buffered score PSUM so exp() on chunk i overlaps score matmuls for
i+1; projections/output-projection get their own PSUM bank pool so the PE
fills exp gaps with projection work; all DRAM I/O is bf16 (host casts);
per-chunk SBUF tiles avoid cross-chunk WAR serialization; softmax reciprocal
uses the fast approx DVE op; causal-mask matmuls are width-trimmed.

Layout tricks kept from baseline:
  - q/k produced transposed [hd, T] via transposed-weight matmuls against xT;
    projection emits lo(0:32)/hi(32:64) half-split channel order so RoPE's
    rotate_half is pure same-partition vector math; small SBUF-SBUF DMAs
    rearrange to per-pair natural head order for the score matmuls.
  - scores computed transposed ([k, q]) so PV consumes probs directly.
  - causal mask = one extra accumulating matmul with ramp constants U, L.
  - softmax max-subtraction skipped (|s*scale| small); scale folded into exp.
  - denominators from ones-weight matmuls landing on the same partitions as
    the attention rows they normalize.
"""

import sys

sys.path.insert(0, "/opt/trn_rl_repo")

from contextlib import ExitStack

import numpy as np
import ml_dtypes

import concourse.bass as bass
import concourse.bacc as bacc
import concourse.tile as tile
import concourse.mybir as mybir
from concourse.bass import ts, ds
from concourse.bass_utils import run_bass_kernel_spmd

B, T, D, H, HD = 2, 2048, 1024, 16, 64
HPC = 4                # heads per core
E = HPC * HD           # 256 per-core channels
W = 256                # attention q-chunk width
NC = T // W            # 8 attention chunks
KT = 128               # k-tile size
GW = 512               # projection/rearrange group width
NG = T // GW           # 4 groups
DQ = D // 128          # 8 contraction subtiles
NEG = -10000.0
FP32 = mybir.dt.float32
BF16 = mybir.dt.bfloat16
SCALE = 1.0 / np.sqrt(HD)


def build_program():
    nc = bacc.Bacc("TRN2", target_bir_lowering=False, debug=False)
    xT_in = nc.declare_dram_parameter("xT_b", [D, T], BF16, isOutput=False)
    wqT = nc.declare_dram_parameter("wqT", [D, E], BF16, isOutput=False)
    wkT = nc.declare_dram_parameter("wkT", [D, E], BF16, isOutput=False)
    wvT = nc.declare_dram_parameter("wvT", [D, E], BF16, isOutput=False)
    wpT = nc.declare_dram_parameter("wpT", [E, D], BF16, isOutput=False)
    cosT = nc.declare_dram_parameter("cosT", [128, T], FP32, isOutput=False)
    sinT = nc.declare_dram_parameter("sinT", [128, T], FP32, isOutput=False)
    umask = nc.declare_dram_parameter("umask", [128, 128], BF16, isOutput=False)
    lmask = nc.declare_dram_parameter("lmask", [128, 512], BF16, isOutput=False)
    lmrep = nc.declare_dram_parameter("lmrep", [128, 1024], BF16, isOutput=False)
    outp = nc.declare_dram_parameter("outp", [T, D], BF16, isOutput=True)

    with tile.TileContext(nc) as tc, ExitStack() as ctx:
        consts = ctx.enter_context(tc.tile_pool(name="consts", bufs=1))
        ropestg = ctx.enter_context(tc.tile_pool(name="ropestg", bufs=2))
        ropetmp = ctx.enter_context(tc.tile_pool(name="ropetmp", bufs=2))
        probs_pool = ctx.enter_context(tc.tile_pool(name="probs", bufs=4))
        recip_pool = ctx.enter_context(tc.tile_pool(name="recip", bufs=2))
        outstage = ctx.enter_context(tc.tile_pool(name="outstage", bufs=2))
        psS = ctx.enter_context(tc.tile_pool(name="psS", bufs=2, space="PSUM"))
        psA = ctx.enter_context(tc.tile_pool(name="psA", bufs=1, space="PSUM"))
        psP = ctx.enter_context(tc.tile_pool(name="psP", bufs=2, space="PSUM"))

        # ---- constants / weights / x to SBUF (all bf16 from host) ----
        # weights first (small, needed by proj(0)); x column-group 0 next so
        # the PE can start within a few us; remaining x groups follow, split
        # across the SWDGE and HWDGE queues.
        xT_sb = consts.tile([128, DQ, T], BF16, tag="xT")
        xT_re = xT_in.rearrange("(o p) m -> p o m", p=128)
        nc.gpsimd.dma_start(xT_sb[:, :, ts(0, GW)], xT_re[:, :, ts(0, GW)])
        w_sb = {}
        for name, w_dram in (("q", wqT), ("k", wkT)):
            w_sb[name] = consts.tile([128, DQ, E], BF16, tag=f"w{name}", name=f"w{name}")
            nc.sync.dma_start(
                w_dst := w_sb[name][:], w_dram.rearrange("(o p) m -> p o m", p=128)
            )
        cos_sb = consts.tile([128, T], FP32, tag="cos")
        sin_sb = consts.tile([128, T], FP32, tag="sin")
        nc.sync.dma_start(cos_sb[:, ts(0, GW)], cosT[:, ts(0, GW)])
        nc.sync.dma_start(sin_sb[:, ts(0, GW)], sinT[:, ts(0, GW)])
        u_sb = consts.tile([128, 128], BF16, tag="umask")
        nc.sync.dma_start(u_sb[:], umask[:])
        lm_sb = consts.tile([128, 512], BF16, tag="lmask")
        nc.sync.dma_start(lm_sb[:], lmask[:])
        lmr_sb = consts.tile([128, 2, 2, W], BF16, tag="lmrep")
        nc.sync.dma_start(lmr_sb[:], lmrep.rearrange("p (x y n) -> p x y n", x=2, y=2))
        w_sb["v"] = consts.tile([128, DQ, E], BF16, tag="wv", name="wv")
        nc.gpsimd.dma_start(
            w_sb["v"][:], wvT.rearrange("(o p) m -> p o m", p=128)
        )
        wp_sb = consts.tile([128, 2, D], BF16, tag="wp")
        nc.gpsimd.dma_start(wp_sb[:], wpT.rearrange("(o p) m -> p o m", p=128))
        for g in range(1, NG):
            nc.gpsimd.dma_start(xT_sb[:, :, ts(g, GW)], xT_re[:, :, ts(g, GW)])

        def prefetch_trig(g):
            nc.sync.dma_start(cos_sb[:, ts(g, GW)], cosT[:, ts(g, GW)])
            nc.sync.dma_start(sin_sb[:, ts(g, GW)], sinT[:, ts(g, GW)])
        ones_sb = consts.tile([128, 64], BF16, tag="ones")
        nc.vector.memset(ones_sb[:], 1.0)
        zer_sb = consts.tile([128, 128], BF16, tag="zer")
        nc.vector.memset(zer_sb[:], 0.0)

        # per-group natural-order roped q/k ([dim1] = pair p: heads 2p,2p+1)
        qn = [consts.tile([128, 2, GW], BF16, tag=f"qn{g}", name=f"qn{g}") for g in range(NG)]
        kn = [consts.tile([128, 2, GW], BF16, tag=f"kn{g}", name=f"kn{g}") for g in range(NG)]
        # per-group v: [t(128), ktile-in-group(4), E]
        vg = [consts.tile([128, 4, E], BF16, tag=f"vg{g}", name=f"vg{g}") for g in range(NG)]
        # per-chunk normalized attention [pair rows, p, W]
        anrm = [consts.tile([128, 2, W], BF16, tag=f"an{c}", name=f"an{c}") for c in range(NC)]

        def proj_qk(g, name, boot=False):
            """q or k projection + RoPE + rearrange for t-cols [GW*g, GW*(g+1))."""
            for name, nat in ((name, qn[g] if name == "q" else kn[g]),):
                if boot:
                    # before attention starts the asum banks are free: run the
                    # boot-phase k projection there so it overlaps q's RoPE
                    pboot = psA.tile([128, 4, W], FP32, tag="asum")
                    ps_hi, ps_lo = pboot[:, 0:2, :], pboot[:, 2:4, :]
                    ps_hi = ps_hi.rearrange("p a b -> p (a b)")
                    ps_lo = ps_lo.rearrange("p a b -> p (a b)")
                else:
                    # two 1-bank psum tiles: lo (channels 0:128), hi (128:256)
                    ps_hi = psP.tile([128, GW], FP32, tag="pp")
                    ps_lo = psP.tile([128, GW], FP32, tag="pp")
                for pdst, half in ((ps_hi, 1), (ps_lo, 0)):
                    for dq in range(DQ):
                        nc.tensor.matmul(
                            pdst[:],
                            lhsT=w_sb[name][:, dq, ds(128 * half, 128)],
                            rhs=xT_sb[:, dq, ts(g, GW)],
                            start=(dq == 0),
                            stop=(dq == DQ - 1),
                        )
                cs, sn = cos_sb[:, ts(g, GW)], sin_sb[:, ts(g, GW)]
                lo_c = ropestg.tile([128, GW], BF16, tag=f"stg{name}lo")
                hi_c = ropestg.tile([128, GW], BF16, tag=f"stg{name}hi")
                t_a = ropetmp.tile([128, GW], BF16, tag="ra")
                t_b = ropetmp.tile([128, GW], BF16, tag="rb")
                t_c = ropetmp.tile([128, GW], BF16, tag="rc")
                t_d = ropetmp.tile([128, GW], BF16, tag="rd")
                nc.vector.tensor_mul(t_a[:], ps_hi[:], sn)
                nc.vector.tensor_mul(t_d[:], ps_hi[:], cs)
                nc.vector.tensor_mul(t_b[:], ps_lo[:], cs)
                nc.vector.tensor_sub(lo_c[:], t_b[:], t_a[:])
                nc.vector.tensor_mul(t_c[:], ps_lo[:], sn)
                nc.vector.tensor_add(hi_c[:], t_d[:], t_c[:])
                # rearrange [4 heads' lo | 4 heads' hi] -> natural per-pair order
                deng = nc.sync
                for h in range(4):
                    p, s = h // 2, h % 2
                    deng.dma_start(
                        nat[ds(64 * s, 32), p, :], lo_c[ds(32 * h, 32), :]
                    )
                    deng.dma_start(
                        nat[ds(64 * s + 32, 32), p, :], hi_c[ds(32 * h, 32), :]
                    )
        def proj_v(g):
            # v for the 4 k-tiles of this group
            for half in range(2):
                psv = psP.tile([128, 2, E], FP32, tag="pp")
                for tt in range(2):
                    t = 4 * g + 2 * half + tt
                    for dq in range(DQ):
                        nc.tensor.matmul(
                            psv[:, tt, :],
                            lhsT=xT_sb[:, dq, ts(t, 128)],
                            rhs=w_sb["v"][:, dq, :],
                            start=(dq == 0),
                            stop=(dq == DQ - 1),
                        )
                nc.vector.tensor_copy(vg[g][:, ds(2 * half, 2), :], psv[:])

        def attn_chunk(c):
            """causal attention for query columns [W*c, W*(c+1))."""
            g_q, cq = c // 2, c % 2
            nk = 2 * c + 2
            # asum: bank0 = attn p0|p1, bank1 = denom p0|p1
            asum = psA.tile([128, 4, W], FP32, tag="asum")
            for bank in range(2):
                nc.tensor.matmul(
                    asum[:, ds(2 * bank, 2), :],
                    lhsT=zer_sb[:],
                    rhs=lmr_sb[:, bank, :, :],
                    start=True,
                    stop=False,
                    skip_group_check=True,
                )
            for i in range(nk):
                g_k, ik = i // 4, i % 4
                j = i - 2 * c  # diag ordinal (0 or 1) when >= 0
                diag = j >= 0
                # scores: [128,2,2,W]: [:, s, p, :] = head 2p+s
                sc = psS.tile([128, 2, 2, W], FP32, tag="sc")
                for p in range(2):
                    for s in range(2):
                        nc.tensor.matmul(
                            sc[:, s, p, :],
                            lhsT=kn[g_k][ds(64 * s, 64), p, ts(ik, KT)],
                            rhs=qn[g_q][ds(64 * s, 64), p, ts(cq, W)],
                            start=(p == 0),
                            stop=(p == 1) and not diag,
                            tile_position=(64 * s, 0),
                            skip_group_check=True,
                        )
                if diag:
                    if j == 1:
                        for s in range(2):
                            nc.tensor.matmul(
                                sc[:, s, :, :],
                                lhsT=u_sb[:],
                                rhs=lmr_sb[:, s, :, :],
                                start=False,
                                stop=True,
                                skip_group_check=True,
                            )
                    else:
                        for s in range(2):
                            for p in range(2):
                                nc.tensor.matmul(
                                    sc[:, s, p, ds(0, 128)],
                                    lhsT=u_sb[:],
                                    rhs=lm_sb[:, ds(384, 128)],
                                    start=False,
                                    stop=(p == 1),
                                    skip_group_check=True,
                                )
                probs = probs_pool.tile([128, 2, 2, W], BF16, tag="probs")
                nc.scalar.activation(
                    probs[:], sc[:], mybir.ActivationFunctionType.Exp, scale=SCALE
                )
                last = i == nk - 1
                for p in range(2):
                    for s in range(2):
                        h = 2 * p + s
                        rows = ds(64 * s, 64)
                        nc.tensor.matmul(
                            asum[rows, p, :],
                            lhsT=vg[g_k][:, ik, ds(64 * h, 64)],
                            rhs=probs[:, s, p, :],
                            start=False,
                            stop=last,
                            tile_position=(0, 64 * s),
                            skip_group_check=True,
                        )
                # denominators: DVE-sum consecutive probs tiles in groups of
                # up to 4 (nk is always even), then one N=512 matmul per side
                # s per group; den[64s+r, 2+p, q] = denom of head 2p+s,
                # matching the row layout of asum[:, p, :] for the normalize.
                if i % 2 == 0:
                    probs_prev = probs
                else:
                    psum2 = probs_pool.tile([128, 2, 2, W], BF16, tag="probs2")
                    nc.vector.tensor_add(psum2[:], probs_prev[:], probs[:])
                    if i % 4 == 1 and not last:
                        psum_hold = psum2
                    else:
                        if i % 4 == 3:
                            psum4 = probs_pool.tile(
                                [128, 2, 2, W], BF16, tag="probs4"
                            )
                            nc.vector.tensor_add(
                                psum4[:], psum_hold[:], psum2[:]
                            )
                            den_rhs = psum4
                        else:
                            den_rhs = psum2
                        for s in range(2):
                            nc.tensor.matmul(
                                asum[ds(64 * s, 64), ds(2, 2), :],
                                lhsT=ones_sb[:],
                                rhs=den_rhs[:, s, :, :],
                                start=False,
                                stop=last,
                                tile_position=(0, 64 * s),
                                skip_group_check=True,
                            )
            rc = recip_pool.tile([128, 2, W], FP32, tag="recip")
            for p in range(2):
                nc.vector.reciprocal_approx_fast(rc[:, p, :], asum[:, 2 + p, :])
                nc.vector.tensor_mul(anrm[c][:, p, :], asum[:, p, :], rc[:, p, :])

        def outproj_chunk(c):
            """output projection for the 2 t-tiles of chunk c."""
            for tt in range(2):
                t = 2 * c + tt
                ost = outstage.tile([128, D], BF16, tag="ost")
                for jj in range(2):
                    po = psP.tile([128, 512], FP32, tag="pp")
                    for p in range(2):
                        nc.tensor.matmul(
                            po[:],
                            lhsT=anrm[c][:, p, ts(tt, 128)],
                            rhs=wp_sb[:, p, ts(jj, 512)],
                            start=(p == 0),
                            stop=(p == 1),
                        )
                    nc.vector.tensor_copy(ost[:, ts(jj, 512)], po[:])
                nc.gpsimd.dma_start(outp[ts(t, 128), :], ost[:])

        # interleave projection pieces between attention chunks so the PE
        # always has independent filler work during exp waits
        proj_qk(0, "q")
        proj_qk(0, "k", boot=True)
        proj_v(0)
        for g in range(NG):
            attn_chunk(2 * g)
            if g + 1 < NG:
                prefetch_trig(g + 1)
                proj_qk(g + 1, "q")
            if g == 3:
                outproj_chunk(2)
                outproj_chunk(3)
            attn_chunk(2 * g + 1)
            if g + 1 < NG:
                proj_qk(g + 1, "k")
                proj_v(g + 1)
            if g == 2:
                outproj_chunk(0)
                outproj_chunk(1)
        for c in (4, 5, 6, 7):
            outproj_chunk(c)

    nc.compile()
    return nc


def host_prep(core, xT_by_batch, Wq, Wk, Wv, Wp, consts):
    b, hp = core // 4, core % 4
    h0 = hp * HPC
    rows = slice(HD * h0, HD * h0 + E)
    bf = ml_dtypes.bfloat16
    Wq_s = np.asarray(Wq[rows]).reshape(HPC, HD, D)
    Wk_s = np.asarray(Wk[rows]).reshape(HPC, HD, D)
    wqT = np.ascontiguousarray(
        np.concatenate(
            [Wq_s[:, :32].reshape(128, D), Wq_s[:, 32:].reshape(128, D)], 0
        ).T
    ).astype(bf)
    wkT = np.ascontiguousarray(
        np.concatenate(
            [Wk_s[:, :32].reshape(128, D), Wk_s[:, 32:].reshape(128, D)], 0
        ).T
    ).astype(bf)
    wvT = np.ascontiguousarray(np.asarray(Wv[rows]).T).astype(bf)
    wpT = np.ascontiguousarray(np.asarray(Wp[:, rows]).T).astype(bf)
    return dict(
        xT_b=xT_by_batch[b],
        wqT=wqT,
        wkT=wkT,
        wvT=wvT,
        wpT=wpT,
        **consts,
    )


def make_consts(cos, sin):
    bf = ml_dtypes.bfloat16
    cosT = np.ascontiguousarray(np.tile(np.asarray(cos[0]).T[:32], (4, 1))).astype(
        np.float32
    )
    sinT = np.ascontiguousarray(np.tile(np.asarray(sin[0]).T[:32], (4, 1))).astype(
        np.float32
    )
    m = np.arange(128)[:, None]
    r = np.arange(128)[None, :]
    umask = np.where(r >= m, NEG, 0.0).astype(bf)
    u_idx = np.arange(512)[None, :]
    lmask = (m >= u_idx - 383).astype(np.float32)
    lmrep = np.tile(lmask[:, 256:512], (1, 4)).astype(bf)
    return dict(
        cosT=cosT,
        sinT=sinT,
        umask=umask,
        lmask=lmask.astype(bf),
        lmrep=lmrep,
    )


_NC_CACHE = None


def _get_nc():
    global _NC_CACHE
    if _NC_CACHE is None:
        _NC_CACHE = build_program()
    return _NC_CACHE


def kernel(x, cos, sin, Wq, Wk, Wv, Wp, _want_trace=False):
    bf = ml_dtypes.bfloat16
    x, cos, sin = np.asarray(x), np.asarray(cos), np.asarray(sin)
    Wq, Wk, Wv, Wp = (np.asarray(a) for a in (Wq, Wk, Wv, Wp))
    nc = _get_nc()
    consts = make_consts(cos, sin)
    xT_by_batch = [np.ascontiguousarray(x[b].T).astype(bf) for b in range(B)]
    in_maps = [
        host_prep(core, xT_by_batch, Wq, Wk, Wv, Wp, consts) for core in range(8)
    ]
    res = run_bass_kernel_spmd(nc, in_maps, list(range(8)), trace=_want_trace)
    out = np.zeros((B, T, D), dtype=np.float32)
    for core in range(8):
        out[core // 4] += np.asarray(res.results[core]["outp"], dtype=np.float32)
    if _want_trace:
        kernel.last_exec_time_ns = res.exec_time_ns
        kernel.last_profile = res.profile_json
    return out


# revision 25
# speedup vs baseline: 1.0044x; 1.0044x over previous
"""Trainium2 Bass kernel for nn_MultiHeadAttention (B=2,T=2048,D=1024,H=16,HD=64).

Sharding: 8 cores = 2 batches x 4 heads/core (tensor parallel over heads).
Each core: q/k/v projections for its 4 heads, RoPE, causal attention, and a
partial output projection (its heads' slice of Wp); host sums 4 partials/batch.

Pipeline structure: attention runs on 256-wide query chunks with a
double-buffered score PSUM (2 banks x2) so exp() of key-tile i overlaps the
score matmuls of i+1; the attention accumulator holds 2 banks and the
projection/output-projection pool the remaining 2, so q/k/v projection and
output-projection matmuls (emitted interleaved between attention chunks, plus
deferred to the tail chunks) keep the PE array dense through every exp wait
and the HAM clock gate stays at full rate. All DRAM I/O is bf16 (host casts,
fp32 partial-sum on host); per-group SBUF tiles avoid cross-chunk WAR
serialization. Score matmuls are 2-way row-packed (tile_position) and PV /
denominator matmuls 2-way column-packed; softmax denominators come from
ones-weight matmuls over DVE-presummed groups of 4 probs tiles; the softmax
reciprocal uses the fast-approx DVE op; causal-mask matmuls are width-trimmed
and merged where a single PSUM bank allows.

Layout tricks kept from baseline:
  - q/k produced transposed [hd, T] via transposed-weight matmuls against xT;
    projection emits lo(0:32)/hi(32:64) half-split channel order so RoPE's
    rotate_half is pure same-partition vector math; small SBUF-SBUF DMAs
    rearrange to per-pair natural head order for the score matmuls.
  - scores computed transposed ([k, q]) so PV consumes probs directly.
  - causal mask = one extra accumulating matmul with ramp constants U, L.
  - softmax max-subtraction skipped (|s*scale| small); scale folded into exp.
  - denominators from ones-weight matmuls landing on the same partitions as
    the attention rows they normalize.
"""

import sys

sys.path.insert(0, "/opt/trn_rl_repo")

from contextlib import ExitStack

import numpy as np
import ml_dtypes

import concourse.bass as bass
import concourse.bacc as bacc
import concourse.tile as tile
import concourse.mybir as mybir
from concourse.bass import ts, ds
from concourse.bass_utils import run_bass_kernel_spmd

B, T, D, H, HD = 2, 2048, 1024, 16, 64
HPC = 4                # heads per core
E = HPC * HD           # 256 per-core channels
W = 256                # attention q-chunk width
NC = T // W            # 8 attention chunks
KT = 128               # k-tile size
GW = 512               # projection/rearrange group width
NG = T // GW           # 4 groups
DQ = D // 128          # 8 contraction subtiles
NEG = -10000.0
FP32 = mybir.dt.float32
BF16 = mybir.dt.bfloat16
SCALE = 1.0 / np.sqrt(HD)


def build_program():
    nc = bacc.Bacc("TRN2", target_bir_lowering=False, debug=False)
    xT_in = nc.declare_dram_parameter("xT_b", [D, T], BF16, isOutput=False)
    wqT = nc.declare_dram_parameter("wqT", [D, E], BF16, isOutput=False)
    wkT = nc.declare_dram_parameter("wkT", [D, E], BF16, isOutput=False)
    wvT = nc.declare_dram_parameter("wvT", [D, E], BF16, isOutput=False)
    wpT = nc.declare_dram_parameter("wpT", [E, D], BF16, isOutput=False)
    cosT = nc.declare_dram_parameter("cosT", [128, T], FP32, isOutput=False)
    sinT = nc.declare_dram_parameter("sinT", [128, T], FP32, isOutput=False)
    umask = nc.declare_dram_parameter("umask", [128, 128], BF16, isOutput=False)
    lmask = nc.declare_dram_parameter("lmask", [128, 512], BF16, isOutput=False)
    lmrep = nc.declare_dram_parameter("lmrep", [128, 1024], BF16, isOutput=False)
    outp = nc.declare_dram_parameter("outp", [T, D], BF16, isOutput=True)

    with tile.TileContext(nc) as tc, ExitStack() as ctx:
        consts = ctx.enter_context(tc.tile_pool(name="consts", bufs=1))
        ropestg = ctx.enter_context(tc.tile_pool(name="ropestg", bufs=2))
        ropetmp = ctx.enter_context(tc.tile_pool(name="ropetmp", bufs=2))
        probs_pool = ctx.enter_context(tc.tile_pool(name="probs", bufs=4))
        recip_pool = ctx.enter_context(tc.tile_pool(name="recip", bufs=2))
        outstage = ctx.enter_context(tc.tile_pool(name="outstage", bufs=2))
        psS = ctx.enter_context(tc.tile_pool(name="psS", bufs=2, space="PSUM"))
        psA = ctx.enter_context(tc.tile_pool(name="psA", bufs=1, space="PSUM"))
        psP = ctx.enter_context(tc.tile_pool(name="psP", bufs=2, space="PSUM"))

        # ---- constants / weights / x to SBUF (all bf16 from host) ----
        # weights first (small, needed by proj(0)); x column-group 0 next so
        # the PE can start within a few us; remaining x groups follow, split
        # across the SWDGE and HWDGE queues.
        xT_sb = consts.tile([128, DQ, T], BF16, tag="xT")
        xT_re = xT_in.rearrange("(o p) m -> p o m", p=128)
        nc.gpsimd.dma_start(xT_sb[:, :, ts(0, GW)], xT_re[:, :, ts(0, GW)])
        w_sb = {}
        for name, w_dram in (("q", wqT), ("k", wkT)):
            w_sb[name] = consts.tile([128, DQ, E], BF16, tag=f"w{name}", name=f"w{name}")
            nc.sync.dma_start(
                w_dst := w_sb[name][:], w_dram.rearrange("(o p) m -> p o m", p=128)
            )
        cos_sb = consts.tile([128, T], FP32, tag="cos")
        sin_sb = consts.tile([128, T], FP32, tag="sin")
        nc.sync.dma_start(cos_sb[:, ts(0, GW)], cosT[:, ts(0, GW)])
        nc.sync.dma_start(sin_sb[:, ts(0, GW)], sinT[:, ts(0, GW)])
        u_sb = consts.tile([128, 128], BF16, tag="umask")
        nc.sync.dma_start(u_sb[:], umask[:])
        lm_sb = consts.tile([128, 512], BF16, tag="lmask")
        nc.sync.dma_start(lm_sb[:], lmask[:])
        lmr_sb = consts.tile([128, 2, 2, W], BF16, tag="lmrep")
        nc.sync.dma_start(lmr_sb[:], lmrep.rearrange("p (x y n) -> p x y n", x=2, y=2))
        w_sb["v"] = consts.tile([128, DQ, E], BF16, tag="wv", name="wv")
        nc.gpsimd.dma_start(
            w_sb["v"][:], wvT.rearrange("(o p) m -> p o m", p=128)
        )
        wp_sb = consts.tile([128, 2, D], BF16, tag="wp")
        nc.gpsimd.dma_start(wp_sb[:], wpT.rearrange("(o p) m -> p o m", p=128))
        for g in range(1, NG):
            nc.gpsimd.dma_start(xT_sb[:, :, ts(g, GW)], xT_re[:, :, ts(g, GW)])

        def prefetch_trig(g):
            nc.sync.dma_start(cos_sb[:, ts(g, GW)], cosT[:, ts(g, GW)])
            nc.sync.dma_start(sin_sb[:, ts(g, GW)], sinT[:, ts(g, GW)])
        ones_sb = consts.tile([128, 64], BF16, tag="ones")
        nc.vector.memset(ones_sb[:], 1.0)
        zer_sb = consts.tile([128, 128], BF16, tag="zer")
        nc.vector.memset(zer_sb[:], 0.0)

        # per-group natural-order roped q/k ([dim1] = pair p: heads 2p,2p+1)
        qn = [consts.tile([128, 2, GW], BF16, tag=f"qn{g}", name=f"qn{g}") for g in range(NG)]
        kn = [consts.tile([128, 2, GW], BF16, tag=f"kn{g}", name=f"kn{g}") for g in range(NG)]
        # per-group v: [t(128), ktile-in-group(4), E]
        vg = [consts.tile([128, 4, E], BF16, tag=f"vg{g}", name=f"vg{g}") for g in range(NG)]
        # per-chunk normalized attention [pair rows, p, W]
        anrm = [consts.tile([128, 2, W], BF16, tag=f"an{c}", name=f"an{c}") for c in range(NC)]

        def proj_qk(g, name, boot=False):
            """q or k projection + RoPE + rearrange for t-cols [GW*g, GW*(g+1))."""
            for name, nat in ((name, qn[g] if name == "q" else kn[g]),):
                if boot:
                    # before attention starts the asum banks are free: run the
                    # boot-phase k projection there so it overlaps q's RoPE
                    pboot = psA.tile([128, 4, W], FP32, tag="asum")
                    ps_hi, ps_lo = pboot[:, 0:2, :], pboot[:, 2:4, :]
                    ps_hi = ps_hi.rearrange("p a b -> p (a b)")
                    ps_lo = ps_lo.rearrange("p a b -> p (a b)")
                else:
                    # two 1-bank psum tiles: lo (channels 0:128), hi (128:256)
                    ps_hi = psP.tile([128, GW], FP32, tag="pp")
                    ps_lo = psP.tile([128, GW], FP32, tag="pp")
                for pdst, half in ((ps_hi, 1), (ps_lo, 0)):
                    for dq in range(DQ):
                        nc.tensor.matmul(
                            pdst[:],
                            lhsT=w_sb[name][:, dq, ds(128 * half, 128)],
                            rhs=xT_sb[:, dq, ts(g, GW)],
                            start=(dq == 0),
                            stop=(dq == DQ - 1),
                        )
                cs, sn = cos_sb[:, ts(g, GW)], sin_sb[:, ts(g, GW)]
                lo_c = ropestg.tile([128, GW], BF16, tag=f"stg{name}lo")
                hi_c = ropestg.tile([128, GW], BF16, tag=f"stg{name}hi")
                t_a = ropetmp.tile([128, GW], BF16, tag="ra")
                t_b = ropetmp.tile([128, GW], BF16, tag="rb")
                t_c = ropetmp.tile([128, GW], BF16, tag="rc")
                t_d = ropetmp.tile([128, GW], BF16, tag="rd")
                nc.vector.tensor_mul(t_a[:], ps_hi[:], sn)
                nc.vector.tensor_mul(t_d[:], ps_hi[:], cs)
                nc.vector.tensor_mul(t_b[:], ps_lo[:], cs)
                nc.vector.tensor_sub(lo_c[:], t_b[:], t_a[:])
                nc.vector.tensor_mul(t_c[:], ps_lo[:], sn)
                nc.vector.tensor_add(hi_c[:], t_d[:], t_c[:])
                # rearrange [4 heads' lo | 4 heads' hi] -> natural per-pair order
                deng = nc.sync
                for h in range(4):
                    p, s = h // 2, h % 2
                    deng.dma_start(
                        nat[ds(64 * s, 32), p, :], lo_c[ds(32 * h, 32), :]
                    )
                    deng.dma_start(
                        nat[ds(64 * s + 32, 32), p, :], hi_c[ds(32 * h, 32), :]
                    )
        def proj_v(g):
            # v for the 4 k-tiles of this group
            for half in range(2):
                psv = psP.tile([128, 2, E], FP32, tag="pp")
                for tt in range(2):
                    t = 4 * g + 2 * half + tt
                    for dq in range(DQ):
                        nc.tensor.matmul(
                            psv[:, tt, :],
                            lhsT=xT_sb[:, dq, ts(t, 128)],
                            rhs=w_sb["v"][:, dq, :],
                            start=(dq == 0),
                            stop=(dq == DQ - 1),
                        )
                nc.vector.tensor_copy(vg[g][:, ds(2 * half, 2), :], psv[:])

        def attn_chunk(c):
            """causal attention for query columns [W*c, W*(c+1))."""
            g_q, cq = c // 2, c % 2
            nk = 2 * c + 2
            # asum: bank0 = attn p0|p1, bank1 = denom p0|p1
            asum = psA.tile([128, 4, W], FP32, tag="asum")
            for bank in range(2):
                nc.tensor.matmul(
                    asum[:, ds(2 * bank, 2), :],
                    lhsT=zer_sb[:],
                    rhs=lmr_sb[:, bank, :, :],
                    start=True,
                    stop=False,
                    skip_group_check=True,
                )
            for i in range(nk):
                g_k, ik = i // 4, i % 4
                j = i - 2 * c  # diag ordinal (0 or 1) when >= 0
                diag = j >= 0
                # scores: [128,2,2,W]: [:, s, p, :] = head 2p+s
                sc = psS.tile([128, 2, 2, W], FP32, tag="sc")
                for p in range(2):
                    for s in range(2):
                        nc.tensor.matmul(
                            sc[:, s, p, :],
                            lhsT=kn[g_k][ds(64 * s, 64), p, ts(ik, KT)],
                            rhs=qn[g_q][ds(64 * s, 64), p, ts(cq, W)],
                            start=(p == 0),
                            stop=(p == 1) and not diag,
                            tile_position=(64 * s, 0),
                            skip_group_check=True,
                        )
                if diag:
                    if j == 1:
                        for s in range(2):
                            nc.tensor.matmul(
                                sc[:, s, :, :],
                                lhsT=u_sb[:],
                                rhs=lmr_sb[:, s, :, :],
                                start=False,
                                stop=True,
                                skip_group_check=True,
                            )
                    else:
                        for s in range(2):
                            for p in range(2):
                                nc.tensor.matmul(
                                    sc[:, s, p, ds(0, 128)],
                                    lhsT=u_sb[:],
                                    rhs=lm_sb[:, ds(384, 128)],
                                    start=False,
                                    stop=(p == 1),
                                    skip_group_check=True,
                                )
                probs = probs_pool.tile([128, 2, 2, W], BF16, tag="probs")
                nc.scalar.activation(
                    probs[:], sc[:], mybir.ActivationFunctionType.Exp, scale=SCALE
                )
                last = i == nk - 1
                for p in range(2):
                    for s in range(2):
                        h = 2 * p + s
                        rows = ds(64 * s, 64)
                        nc.tensor.matmul(
                            asum[rows, p, :],
                            lhsT=vg[g_k][:, ik, ds(64 * h, 64)],
                            rhs=probs[:, s, p, :],
                            start=False,
                            stop=last,
                            tile_position=(0, 64 * s),
                            skip_group_check=True,
                        )
                # denominators: DVE-sum consecutive probs tiles in groups of
                # up to 4 (nk is always even), then one N=512 matmul per side
                # s per group; den[64s+r, 2+p, q] = denom of head 2p+s,
                # matching the row layout of asum[:, p, :] for the normalize.
                if i % 2 == 0:
                    probs_prev = probs
                else:
                    psum2 = probs_pool.tile([128, 2, 2, W], BF16, tag="probs2")
                    nc.vector.tensor_add(psum2[:], probs_prev[:], probs[:])
                    if i % 4 == 1 and not last:
                        psum_hold = psum2
                    else:
                        if i % 4 == 3:
                            psum4 = probs_pool.tile(
                                [128, 2, 2, W], BF16, tag="probs4"
                            )
                            nc.vector.tensor_add(
                                psum4[:], psum_hold[:], psum2[:]
                            )
                            den_rhs = psum4
                        else:
                            den_rhs = psum2
                        for s in range(2):
                            nc.tensor.matmul(
                                asum[ds(64 * s, 64), ds(2, 2), :],
                                lhsT=ones_sb[:],
                                rhs=den_rhs[:, s, :, :],
                                start=False,
                                stop=last,
                                tile_position=(0, 64 * s),
                                skip_group_check=True,
                            )
            rc = recip_pool.tile([128, 2, W], FP32, tag="recip")
            for p in range(2):
                nc.vector.reciprocal_approx_fast(rc[:, p, :], asum[:, 2 + p, :])
                nc.vector.tensor_mul(anrm[c][:, p, :], asum[:, p, :], rc[:, p, :])

        def outproj_chunk(c):
            """output projection for the 2 t-tiles of chunk c."""
            for tt in range(2):
                t = 2 * c + tt
                ost = outstage.tile([128, D], BF16, tag="ost")
                for jj in range(2):
                    po = psP.tile([128, 512], FP32, tag="pp")
                    for p in range(2):
                        nc.tensor.matmul(
                            po[:],
                            lhsT=anrm[c][:, p, ts(tt, 128)],
                            rhs=wp_sb[:, p, ts(jj, 512)],
                            start=(p == 0),
                            stop=(p == 1),
                        )
                    nc.vector.tensor_copy(ost[:, ts(jj, 512)], po[:])
                nc.gpsimd.dma_start(outp[ts(t, 128), :], ost[:])

        # interleave projection pieces between attention chunks so the PE
        # always has independent filler work during exp waits
        proj_qk(0, "q")
        proj_qk(0, "k", boot=True)
        proj_v(0)
        for g in range(NG):
            attn_chunk(2 * g)
            if g + 1 < NG:
                prefetch_trig(g + 1)
                proj_qk(g + 1, "q")
            if g == 3:
                outproj_chunk(2)
                outproj_chunk(3)
            attn_chunk(2 * g + 1)
            if g + 1 < NG:
                proj_qk(g + 1, "k")
                proj_v(g + 1)
            if g == 2:
                outproj_chunk(0)
                outproj_chunk(1)
        for c in (4, 5, 6, 7):
            outproj_chunk(c)

    nc.compile()
    return nc


def host_prep(core, xT_by_batch, Wq, Wk, Wv, Wp, consts):
    b, hp = core // 4, core % 4
    h0 = hp * HPC
    rows = slice(HD * h0, HD * h0 + E)
    bf = ml_dtypes.bfloat16
    Wq_s = np.asarray(Wq[rows]).reshape(HPC, HD, D)
    Wk_s = np.asarray(Wk[rows]).reshape(HPC, HD, D)
    wqT = np.ascontiguousarray(
        np.concatenate(
            [Wq_s[:, :32].reshape(128, D), Wq_s[:, 32:].reshape(128, D)], 0
        ).T
    ).astype(bf)
    wkT = np.ascontiguousarray(
        np.concatenate(
            [Wk_s[:, :32].reshape(128, D), Wk_s[:, 32:].reshape(128, D)], 0
        ).T
    ).astype(bf)
    wvT = np.ascontiguousarray(np.asarray(Wv[rows]).T).astype(bf)
    wpT = np.ascontiguousarray(np.asarray(Wp[:, rows]).T).astype(bf)
    return dict(
        xT_b=xT_by_batch[b],
        wqT=wqT,
        wkT=wkT,
        wvT=wvT,
        wpT=wpT,
        **consts,
    )


def make_consts(cos, sin):
    bf = ml_dtypes.bfloat16
    cosT = np.ascontiguousarray(np.tile(np.asarray(cos[0]).T[:32], (4, 1))).astype(
        np.float32
    )
    sinT = np.ascontiguousarray(np.tile(np.asarray(sin[0]).T[:32], (4, 1))).astype(
        np.float32
    )
    m = np.arange(128)[:, None]
    r = np.arange(128)[None, :]
    umask = np.where(r >= m, NEG, 0.0).astype(bf)
    u_idx = np.arange(512)[None, :]
    lmask = (m >= u_idx - 383).astype(np.float32)
    lmrep = np.tile(lmask[:, 256:512], (1, 4)).astype(bf)
    return dict(
        cosT=cosT,
        sinT=sinT,
        umask=umask,
        lmask=lmask.astype(bf),
        lmrep=lmrep,
    )


_NC_CACHE = None


def _get_nc():
    global _NC_CACHE
    if _NC_CACHE is None:
        _NC_CACHE = build_program()
    return _NC_CACHE


def kernel(x, cos, sin, Wq, Wk, Wv, Wp, _want_trace=False):
    bf = ml_dtypes.bfloat16
    x, cos, sin = np.asarray(x), np.asarray(cos), np.asarray(sin)
    Wq, Wk, Wv, Wp = (np.asarray(a) for a in (Wq, Wk, Wv, Wp))
    nc = _get_nc()
    consts = make_consts(cos, sin)
    xT_by_batch = [np.ascontiguousarray(x[b].T).astype(bf) for b in range(B)]
    in_maps = [
        host_prep(core, xT_by_batch, Wq, Wk, Wv, Wp, consts) for core in range(8)
    ]
    res = run_bass_kernel_spmd(nc, in_maps, list(range(8)), trace=_want_trace)
    out = np.zeros((B, T, D), dtype=np.float32)
    for core in range(8):
        out[core // 4] += np.asarray(res.results[core]["outp"], dtype=np.float32)
    if _want_trace:
        kernel.last_exec_time_ns = res.exec_time_ns
        kernel.last_profile = res.profile_json
    return out


# revision 26
# speedup vs baseline: 1.0212x; 1.0166x over previous
"""Trainium2 Bass kernel for nn_MultiHeadAttention (B=2,T=2048,D=1024,H=16,HD=64).

Sharding: 8 cores = 2 batches x 4 heads/core (tensor parallel over heads).
Each core: q/k/v projections for its 4 heads, RoPE, causal attention, and a
partial output projection (its heads' slice of Wp); host sums 4 partials/batch.

Pipeline structure: attention runs on 256-wide query chunks with a
double-buffered score PSUM (2 banks x2) so exp() of key-tile i overlaps the
score matmuls of i+1; the attention accumulator holds 2 banks and the
projection/output-projection pool the remaining 2, so q/k/v projection and
output-projection matmuls (emitted interleaved between attention chunks, plus
deferred to the tail chunks) keep the PE array dense through every exp wait
and the HAM clock gate stays at full rate. All DRAM I/O is bf16 (host casts,
fp32 partial-sum on host); per-group SBUF tiles avoid cross-chunk WAR
serialization. Score matmuls are 2-way row-packed (tile_position) and PV /
denominator matmuls 2-way column-packed; softmax denominators come from
ones-weight matmuls over DVE-presummed groups of 4 probs tiles; the softmax
reciprocal uses the fast-approx DVE op; causal-mask matmuls are width-trimmed
and merged where a single PSUM bank allows.

Layout tricks kept from baseline:
  - q/k produced transposed [hd, T] via transposed-weight matmuls against xT;
    projection emits lo(0:32)/hi(32:64) half-split channel order so RoPE's
    rotate_half is pure same-partition vector math; small SBUF-SBUF DMAs
    rearrange to per-pair natural head order for the score matmuls.
  - scores computed transposed ([k, q]) so PV consumes probs directly.
  - causal mask = one extra accumulating matmul with ramp constants U, L.
  - softmax max-subtraction skipped (|s*scale| small); scale folded into exp.
  - denominators from ones-weight matmuls landing on the same partitions as
    the attention rows they normalize.
"""

import sys

sys.path.insert(0, "/opt/trn_rl_repo")

from contextlib import ExitStack

import numpy as np
import ml_dtypes

import concourse.bass as bass
import concourse.bacc as bacc
import concourse.tile as tile
import concourse.mybir as mybir
from concourse.bass import ts, ds
from concourse.bass_utils import run_bass_kernel_spmd

B, T, D, H, HD = 2, 2048, 1024, 16, 64
HPC = 4                # heads per core
E = HPC * HD           # 256 per-core channels
W = 256                # attention q-chunk width
NC = T // W            # 8 attention chunks
KT = 128               # k-tile size
GW = 512               # projection/rearrange group width
NG = T // GW           # 4 groups
DQ = D // 128          # 8 contraction subtiles
NEG = -10000.0
FP32 = mybir.dt.float32
BF16 = mybir.dt.bfloat16
SCALE = 1.0 / np.sqrt(HD)


def build_program():
    nc = bacc.Bacc("TRN2", target_bir_lowering=False, debug=False)
    xT_in = nc.declare_dram_parameter("xT_b", [D, T], BF16, isOutput=False)
    wqT = nc.declare_dram_parameter("wqT", [D, E], BF16, isOutput=False)
    wkT = nc.declare_dram_parameter("wkT", [D, E], BF16, isOutput=False)
    wvT = nc.declare_dram_parameter("wvT", [D, E], BF16, isOutput=False)
    wpT = nc.declare_dram_parameter("wpT", [E, D], BF16, isOutput=False)
    cosT = nc.declare_dram_parameter("cosT", [128, T], FP32, isOutput=False)
    sinT = nc.declare_dram_parameter("sinT", [128, T], FP32, isOutput=False)
    umask = nc.declare_dram_parameter("umask", [128, 128], BF16, isOutput=False)
    lmask = nc.declare_dram_parameter("lmask", [128, 512], BF16, isOutput=False)
    lmrep = nc.declare_dram_parameter("lmrep", [128, 1024], BF16, isOutput=False)
    outp = nc.declare_dram_parameter("outp", [T, D], BF16, isOutput=True)

    with tile.TileContext(nc) as tc, ExitStack() as ctx:
        consts = ctx.enter_context(tc.tile_pool(name="consts", bufs=1))
        ropestg = ctx.enter_context(tc.tile_pool(name="ropestg", bufs=2))
        ropetmp = ctx.enter_context(tc.tile_pool(name="ropetmp", bufs=2))
        probs_pool = ctx.enter_context(tc.tile_pool(name="probs", bufs=4))
        recip_pool = ctx.enter_context(tc.tile_pool(name="recip", bufs=2))
        outstage = ctx.enter_context(tc.tile_pool(name="outstage", bufs=2))
        psS = ctx.enter_context(tc.tile_pool(name="psS", bufs=2, space="PSUM"))
        psA = ctx.enter_context(tc.tile_pool(name="psA", bufs=1, space="PSUM"))
        psP = ctx.enter_context(tc.tile_pool(name="psP", bufs=2, space="PSUM"))

        # ---- constants / weights / x to SBUF (all bf16 from host) ----
        # weights first (small, needed by proj(0)); x column-group 0 next so
        # the PE can start within a few us; remaining x groups follow, split
        # across the SWDGE and HWDGE queues.
        xT_sb = consts.tile([128, DQ, T], BF16, tag="xT")
        xT_re = xT_in.rearrange("(o p) m -> p o m", p=128)
        nc.gpsimd.dma_start(xT_sb[:, :, ts(0, GW)], xT_re[:, :, ts(0, GW)])
        w_sb = {}
        for name, w_dram in (("q", wqT), ("k", wkT)):
            w_sb[name] = consts.tile([128, DQ, E], BF16, tag=f"w{name}", name=f"w{name}")
            nc.sync.dma_start(
                w_dst := w_sb[name][:], w_dram.rearrange("(o p) m -> p o m", p=128)
            )
        cos_sb = consts.tile([128, T], FP32, tag="cos")
        sin_sb = consts.tile([128, T], FP32, tag="sin")
        nc.sync.dma_start(cos_sb[:, ts(0, GW)], cosT[:, ts(0, GW)])
        nc.sync.dma_start(sin_sb[:, ts(0, GW)], sinT[:, ts(0, GW)])
        u_sb = consts.tile([128, 128], BF16, tag="umask")
        nc.sync.dma_start(u_sb[:], umask[:])
        lm_sb = consts.tile([128, 512], BF16, tag="lmask")
        nc.sync.dma_start(lm_sb[:], lmask[:])
        lmr_sb = consts.tile([128, 2, 2, W], BF16, tag="lmrep")
        nc.sync.dma_start(lmr_sb[:], lmrep.rearrange("p (x y n) -> p x y n", x=2, y=2))
        w_sb["v"] = consts.tile([128, DQ, E], BF16, tag="wv", name="wv")
        nc.gpsimd.dma_start(
            w_sb["v"][:], wvT.rearrange("(o p) m -> p o m", p=128)
        )
        wp_sb = consts.tile([128, 2, D], BF16, tag="wp")
        nc.gpsimd.dma_start(wp_sb[:], wpT.rearrange("(o p) m -> p o m", p=128))
        for g in range(1, NG):
            nc.gpsimd.dma_start(xT_sb[:, :, ts(g, GW)], xT_re[:, :, ts(g, GW)])

        def prefetch_trig(g):
            nc.sync.dma_start(cos_sb[:, ts(g, GW)], cosT[:, ts(g, GW)])
            nc.sync.dma_start(sin_sb[:, ts(g, GW)], sinT[:, ts(g, GW)])
        ones_sb = consts.tile([128, 64], BF16, tag="ones")
        nc.vector.memset(ones_sb[:], 1.0)
        zer_sb = consts.tile([128, 128], BF16, tag="zer")
        nc.vector.memset(zer_sb[:], 0.0)

        # per-group natural-order roped q/k ([dim1] = pair p: heads 2p,2p+1)
        qn = [consts.tile([128, 2, GW], BF16, tag=f"qn{g}", name=f"qn{g}") for g in range(NG)]
        kn = [consts.tile([128, 2, GW], BF16, tag=f"kn{g}", name=f"kn{g}") for g in range(NG)]
        # per-group v: [t(128), ktile-in-group(4), E]
        vg = [consts.tile([128, 4, E], BF16, tag=f"vg{g}", name=f"vg{g}") for g in range(NG)]
        # per-chunk normalized attention [pair rows, p, W]
        anrm = [consts.tile([128, 2, W], BF16, tag=f"an{c}", name=f"an{c}") for c in range(NC)]

        def proj_qk(g, name, boot=False):
            """q or k projection + RoPE + rearrange for t-cols [GW*g, GW*(g+1))."""
            for name, nat in ((name, qn[g] if name == "q" else kn[g]),):
                if boot:
                    # before attention starts the asum banks are free: run the
                    # boot-phase k projection there so it overlaps q's RoPE
                    pboot = psA.tile([128, 4, W], FP32, tag="asum")
                    ps_hi, ps_lo = pboot[:, 0:2, :], pboot[:, 2:4, :]
                    ps_hi = ps_hi.rearrange("p a b -> p (a b)")
                    ps_lo = ps_lo.rearrange("p a b -> p (a b)")
                else:
                    # two 1-bank psum tiles: lo (channels 0:128), hi (128:256)
                    ps_hi = psP.tile([128, GW], FP32, tag="pp")
                    ps_lo = psP.tile([128, GW], FP32, tag="pp")
                for pdst, half in ((ps_hi, 1), (ps_lo, 0)):
                    for dq in range(DQ):
                        nc.tensor.matmul(
                            pdst[:],
                            lhsT=w_sb[name][:, dq, ds(128 * half, 128)],
                            rhs=xT_sb[:, dq, ts(g, GW)],
                            start=(dq == 0),
                            stop=(dq == DQ - 1),
                        )
                cs, sn = cos_sb[:, ts(g, GW)], sin_sb[:, ts(g, GW)]
                lo_c = ropestg.tile([128, GW], BF16, tag=f"stg{name}lo")
                hi_c = ropestg.tile([128, GW], BF16, tag=f"stg{name}hi")
                t_a = ropetmp.tile([128, GW], BF16, tag="ra")
                t_b = ropetmp.tile([128, GW], BF16, tag="rb")
                t_c = ropetmp.tile([128, GW], BF16, tag="rc")
                t_d = ropetmp.tile([128, GW], BF16, tag="rd")
                nc.vector.tensor_mul(t_a[:], ps_hi[:], sn)
                nc.vector.tensor_mul(t_d[:], ps_hi[:], cs)
                nc.vector.tensor_mul(t_b[:], ps_lo[:], cs)
                nc.vector.tensor_sub(lo_c[:], t_b[:], t_a[:])
                # rearrange [4 heads' lo | 4 heads' hi] -> natural per-pair
                # order; lo DMAs issue while the hi half is still in RoPE, and
                # the 16 transfers split across the HWDGE and SWDGE queues.
                for h in range(4):
                    p, s = h // 2, h % 2
                    eng = nc.sync if h % 2 == 0 else nc.gpsimd
                    eng.dma_start(
                        nat[ds(64 * s, 32), p, :], lo_c[ds(32 * h, 32), :]
                    )
                nc.vector.tensor_mul(t_c[:], ps_lo[:], sn)
                nc.vector.tensor_add(hi_c[:], t_d[:], t_c[:])
                for h in range(4):
                    p, s = h // 2, h % 2
                    eng = nc.sync if h % 2 == 1 else nc.gpsimd
                    eng.dma_start(
                        nat[ds(64 * s + 32, 32), p, :], hi_c[ds(32 * h, 32), :]
                    )
        def proj_v(g):
            # v for the 4 k-tiles of this group
            for half in range(2):
                psv = psP.tile([128, 2, E], FP32, tag="pp")
                for tt in range(2):
                    t = 4 * g + 2 * half + tt
                    for dq in range(DQ):
                        nc.tensor.matmul(
                            psv[:, tt, :],
                            lhsT=xT_sb[:, dq, ts(t, 128)],
                            rhs=w_sb["v"][:, dq, :],
                            start=(dq == 0),
                            stop=(dq == DQ - 1),
                        )
                nc.vector.tensor_copy(vg[g][:, ds(2 * half, 2), :], psv[:])

        def attn_chunk(c):
            """causal attention for query columns [W*c, W*(c+1))."""
            g_q, cq = c // 2, c % 2
            nk = 2 * c + 2
            # asum: bank0 = attn p0|p1, bank1 = denom p0|p1
            asum = psA.tile([128, 4, W], FP32, tag="asum")
            for bank in range(2):
                nc.tensor.matmul(
                    asum[:, ds(2 * bank, 2), :],
                    lhsT=zer_sb[:],
                    rhs=lmr_sb[:, bank, :, :],
                    start=True,
                    stop=False,
                    skip_group_check=True,
                )
            for i in range(nk):
                g_k, ik = i // 4, i % 4
                j = i - 2 * c  # diag ordinal (0 or 1) when >= 0
                diag = j >= 0
                # scores: [128,2,2,W]: [:, s, p, :] = head 2p+s
                sc = psS.tile([128, 2, 2, W], FP32, tag="sc")
                for p in range(2):
                    for s in range(2):
                        nc.tensor.matmul(
                            sc[:, s, p, :],
                            lhsT=kn[g_k][ds(64 * s, 64), p, ts(ik, KT)],
                            rhs=qn[g_q][ds(64 * s, 64), p, ts(cq, W)],
                            start=(p == 0),
                            stop=(p == 1) and not diag,
                            tile_position=(64 * s, 0),
                            skip_group_check=True,
                        )
                if diag:
                    if j == 1:
                        for s in range(2):
                            nc.tensor.matmul(
                                sc[:, s, :, :],
                                lhsT=u_sb[:],
                                rhs=lmr_sb[:, s, :, :],
                                start=False,
                                stop=True,
                                skip_group_check=True,
                            )
                    else:
                        for s in range(2):
                            for p in range(2):
                                nc.tensor.matmul(
                                    sc[:, s, p, ds(0, 128)],
                                    lhsT=u_sb[:],
                                    rhs=lm_sb[:, ds(384, 128)],
                                    start=False,
                                    stop=(p == 1),
                                    skip_group_check=True,
                                )
                probs = probs_pool.tile([128, 2, 2, W], BF16, tag="probs")
                nc.scalar.activation(
                    probs[:], sc[:], mybir.ActivationFunctionType.Exp, scale=SCALE
                )
                last = i == nk - 1
                for p in range(2):
                    for s in range(2):
                        h = 2 * p + s
                        rows = ds(64 * s, 64)
                        nc.tensor.matmul(
                            asum[rows, p, :],
                            lhsT=vg[g_k][:, ik, ds(64 * h, 64)],
                            rhs=probs[:, s, p, :],
                            start=False,
                            stop=last,
                            tile_position=(0, 64 * s),
                            skip_group_check=True,
                        )
                # denominators: DVE-sum consecutive probs tiles in groups of
                # up to 4 (nk is always even), then one N=512 matmul per side
                # s per group; den[64s+r, 2+p, q] = denom of head 2p+s,
                # matching the row layout of asum[:, p, :] for the normalize.
                if i % 2 == 0:
                    probs_prev = probs
                else:
                    psum2 = probs_pool.tile([128, 2, 2, W], BF16, tag="probs2")
                    nc.vector.tensor_add(psum2[:], probs_prev[:], probs[:])
                    if i % 4 == 1 and not last:
                        psum_hold = psum2
                    else:
                        if i % 4 == 3:
                            psum4 = probs_pool.tile(
                                [128, 2, 2, W], BF16, tag="probs4"
                            )
                            nc.vector.tensor_add(
                                psum4[:], psum_hold[:], psum2[:]
                            )
                            den_rhs = psum4
                        else:
                            den_rhs = psum2
                        for s in range(2):
                            nc.tensor.matmul(
                                asum[ds(64 * s, 64), ds(2, 2), :],
                                lhsT=ones_sb[:],
                                rhs=den_rhs[:, s, :, :],
                                start=False,
                                stop=last,
                                tile_position=(0, 64 * s),
                                skip_group_check=True,
                            )
            rc = recip_pool.tile([128, 2, W], FP32, tag="recip")
            for p in range(2):
                nc.vector.reciprocal_approx_fast(rc[:, p, :], asum[:, 2 + p, :])
                nc.vector.tensor_mul(anrm[c][:, p, :], asum[:, p, :], rc[:, p, :])

        def outproj_chunk(c):
            """output projection for the 2 t-tiles of chunk c."""
            for tt in range(2):
                t = 2 * c + tt
                ost = outstage.tile([128, D], BF16, tag="ost")
                for jj in range(2):
                    po = psP.tile([128, 512], FP32, tag="pp")
                    for p in range(2):
                        nc.tensor.matmul(
                            po[:],
                            lhsT=anrm[c][:, p, ts(tt, 128)],
                            rhs=wp_sb[:, p, ts(jj, 512)],
                            start=(p == 0),
                            stop=(p == 1),
                        )
                    nc.vector.tensor_copy(ost[:, ts(jj, 512)], po[:])
                nc.gpsimd.dma_start(outp[ts(t, 128), :], ost[:])

        # interleave projection pieces between attention chunks so the PE
        # always has independent filler work during exp waits
        proj_qk(0, "q")
        proj_qk(0, "k", boot=True)
        proj_v(0)
        for g in range(NG):
            attn_chunk(2 * g)
            if g + 1 < NG:
                prefetch_trig(g + 1)
                proj_qk(g + 1, "q")
            if g == 3:
                outproj_chunk(2)
                outproj_chunk(3)
            attn_chunk(2 * g + 1)
            if g + 1 < NG:
                proj_qk(g + 1, "k")
                proj_v(g + 1)
            if g == 2:
                outproj_chunk(0)
                outproj_chunk(1)
        for c in (4, 5, 6, 7):
            outproj_chunk(c)

    nc.compile()
    return nc


def host_prep(core, xT_by_batch, Wq, Wk, Wv, Wp, consts):
    b, hp = core // 4, core % 4
    h0 = hp * HPC
    rows = slice(HD * h0, HD * h0 + E)
    bf = ml_dtypes.bfloat16
    Wq_s = np.asarray(Wq[rows]).reshape(HPC, HD, D)
    Wk_s = np.asarray(Wk[rows]).reshape(HPC, HD, D)
    wqT = np.ascontiguousarray(
        np.concatenate(
            [Wq_s[:, :32].reshape(128, D), Wq_s[:, 32:].reshape(128, D)], 0
        ).T
    ).astype(bf)
    wkT = np.ascontiguousarray(
        np.concatenate(
            [Wk_s[:, :32].reshape(128, D), Wk_s[:, 32:].reshape(128, D)], 0
        ).T
    ).astype(bf)
    wvT = np.ascontiguousarray(np.asarray(Wv[rows]).T).astype(bf)
    wpT = np.ascontiguousarray(np.asarray(Wp[:, rows]).T).astype(bf)
    return dict(
        xT_b=xT_by_batch[b],
        wqT=wqT,
        wkT=wkT,
        wvT=wvT,
        wpT=wpT,
        **consts,
    )


def make_consts(cos, sin):
    bf = ml_dtypes.bfloat16
    cosT = np.ascontiguousarray(np.tile(np.asarray(cos[0]).T[:32], (4, 1))).astype(
        np.float32
    )
    sinT = np.ascontiguousarray(np.tile(np.asarray(sin[0]).T[:32], (4, 1))).astype(
        np.float32
    )
    m = np.arange(128)[:, None]
    r = np.arange(128)[None, :]
    umask = np.where(r >= m, NEG, 0.0).astype(bf)
    u_idx = np.arange(512)[None, :]
    lmask = (m >= u_idx - 383).astype(np.float32)
    lmrep = np.tile(lmask[:, 256:512], (1, 4)).astype(bf)
    return dict(
        cosT=cosT,
        sinT=sinT,
        umask=umask,
        lmask=lmask.astype(bf),
        lmrep=lmrep,
    )


_NC_CACHE = None


def _get_nc():
    global _NC_CACHE
    if _NC_CACHE is None:
        _NC_CACHE = build_program()
    return _NC_CACHE


def kernel(x, cos, sin, Wq, Wk, Wv, Wp, _want_trace=False):
    bf = ml_dtypes.bfloat16
    x, cos, sin = np.asarray(x), np.asarray(cos), np.asarray(sin)
    Wq, Wk, Wv, Wp = (np.asarray(a) for a in (Wq, Wk, Wv, Wp))
    nc = _get_nc()
    consts = make_consts(cos, sin)
    xT_by_batch = [np.ascontiguousarray(x[b].T).astype(bf) for b in range(B)]
    in_maps = [
        host_prep(core, xT_by_batch, Wq, Wk, Wv, Wp, consts) for core in range(8)
    ]
    res = run_bass_kernel_spmd(nc, in_maps, list(range(8)), trace=_want_trace)
    out = np.zeros((B, T, D), dtype=np.float32)
    for core in range(8):
        out[core // 4] += np.asarray(res.results[core]["outp"], dtype=np.float32)
    if _want_trace:
        kernel.last_exec_time_ns = res.exec_time_ns
        kernel.last_profile = res.profile_json
    return out
